# revision 1
# baseline (speedup 1.0000x reference)
"""XNOR-Net++ 3x3 conv (sign(x) (*) sign(w) * alpha*beta*gamma) on 8 TRN2 NeuronCores.

Sharding: data-parallel over batch (32 -> 4 per core), weights/scales replicated.

Per core (measured 176 us HW exec, exact vs fp32 reference):
- binarize x and w on-device to fp8e4 (+-1 is exact; PSUM accumulates fp32 exactly)
- sign images stored as three x-shifted contiguous fp8 copies (one per kx tap),
  each [128, 2, 58, 56], so the DoubleRow rhs AP is exactly [K=128, 2, N=448]
- 3x3 conv = 9 accumulating DoubleRow matmuls per [128, 448] output tile
  (K=256 via input-channel-block pairing, 2 fp8 weights/PE cell)
- weights transposed on-device via PE transpose; pair dim step 128 B (%16==0)
- epilogue: alpha per-channel scale on ACT, beta*gamma per-pixel map on DVE
"""

from contextlib import ExitStack

import numpy as np

import concourse.bacc as bacc
import concourse.bass as bass
import concourse.mybir as mybir
import concourse.tile as tile
from concourse import masks
from concourse.bass_utils import run_bass_kernel_spmd

N_CORES = 8
B, C, H, KS = 32, 256, 56, 3
P = 128
CB = C // P  # input-channel blocks (2)
OB = C // P  # output-channel blocks (2)
HP = H + 2   # padded image rows (58)
R = 8        # output rows per matmul tile
T = H // R   # row tiles per image (7)
NT = R * H   # moving free dim per matmul (448)
HW = H * H   # pixels per image (3136)

F32 = mybir.dt.float32
BF16 = mybir.dt.bfloat16
FP8 = mybir.dt.float8e4
DR = mybir.MatmulPerfMode.DoubleRow


def build_conv(tc, out_ap, x_ap, w_ap, a_ap, b_ap, g_ap, BL):
    nc = tc.nc
    with ExitStack() as ctx:
        const_pool = ctx.enter_context(tc.tile_pool(name="const", bufs=1))
        wpool = ctx.enter_context(tc.tile_pool(name="w", bufs=1))
        xpool = ctx.enter_context(tc.tile_pool(name="x", bufs=2))
        imgpool = ctx.enter_context(tc.tile_pool(name="img", bufs=2))
        psumpool = ctx.enter_context(tc.tile_pool(name="psum", bufs=4, space="PSUM"))
        tpool = ctx.enter_context(tc.tile_pool(name="tmp", bufs=4))
        opool = ctx.enter_context(tc.tile_pool(name="o", bufs=4))

        ident = const_pool.tile([P, P], BF16, name="ident")
        masks.make_identity(nc, ident)

        # ---- weights: load, binarize, transpose, convert to fp8 ----
        w_f32 = wpool.tile([P, OB, C * KS * KS], F32, name="w_f32")
        nc.sync.dma_start(
            w_f32, w_ap.rearrange("(ob p) i ky kx -> p ob (i ky kx)", p=P)
        )
        w_sgn = wpool.tile([P, OB, C * KS * KS], BF16, name="w_sgn")
        nc.scalar.sign(w_sgn, w_f32)
        w_view = w_sgn.rearrange("p ob (i kk) -> p ob kk i", kk=KS * KS)

        # wT2[i_low, tap, ob, cb, o] in fp8; pair dim cb has byte-step 128 (%16==0)
        wT2 = wpool.tile([P, KS * KS, OB, CB, P], FP8, name="wT2")
        for ob in range(OB):
            for ib in range(CB):
                for kk in range(KS * KS):
                    pt = psumpool.tile([P, P], BF16, name="pt", tag="pt", bufs=2)
                    nc.tensor.transpose(
                        pt, w_view[:, ob, kk, ib * P : (ib + 1) * P], ident
                    )
                    nc.scalar.copy(wT2[:, kk, ob, ib, :], pt)

        # ---- scales ----
        a_t = const_pool.tile([P, OB], F32, name="a_t")
        nc.sync.dma_start(a_t, a_ap.rearrange("(ob p) u v -> p (ob u v)", p=P))
        b_t = const_pool.tile([1, H], F32, name="b_t")
        nc.sync.dma_start(b_t, b_ap[0:1, :, 0])
        g_t = const_pool.tile([1, H], F32, name="g_t")
        nc.sync.dma_start(g_t, g_ap[0:1, 0, :])

        # bg_row[0, i*56+j] = beta[i] * gamma[j] — one DVE op, step-0 broadcast reads
        bg_row = const_pool.tile([1, HW], F32, name="bg_row")
        b_rep = b_t[0:1, :].unsqueeze(2).to_broadcast((1, H, H))
        g_rep = g_t[0:1, :].unsqueeze(1).to_broadcast((1, H, H))
        nc.vector.tensor_mul(bg_row.rearrange("a (i j) -> a i j", i=H), b_rep, g_rep)
        ones_t = const_pool.tile([1, P], F32, name="ones_t")
        nc.gpsimd.memset(ones_t, 1.0)
        # broadcast to all 128 partitions via K=1 matmul
        bg_bcast = const_pool.tile([P, HW], F32, name="bg_bcast")
        for t in range(T):
            sl = slice(t * NT, (t + 1) * NT)
            bgp = psumpool.tile([P, NT], F32, name="bgp", tag="bgp", bufs=2)
            nc.tensor.matmul(bgp, ones_t, bg_row[0:1, sl], start=True, stop=True)
            nc.scalar.copy(bg_bcast[:, sl], bgp)

        # ---- main loop over local batches ----
        x_v = x_ap.rearrange("b (cb p) h w -> b p cb (h w)", p=P)
        out_v = out_ap.rearrange("b (ob p) h w -> b ob p (h w)", p=P)
        for b in range(BL):
            x_t = xpool.tile([P, CB, HW], F32, name="x_t")
            nc.sync.dma_start(x_t, x_v[b])
            # im[kx][p, cb, y, j] = padded_sign[p, cb, y, j + kx]
            im1 = imgpool.tile([P, CB, HP, H], FP8, name="im1", tag="im1")
            im0 = imgpool.tile([P, CB, HP, H], FP8, name="im0", tag="im0")
            im2 = imgpool.tile([P, CB, HP, H], FP8, name="im2", tag="im2")
            nc.gpsimd.memset(im1, 0.0)
            nc.gpsimd.memset(im0, 0.0)
            nc.gpsimd.memset(im2, 0.0)
            # kx=1: no column shift — interior rows get the full sign image
            nc.scalar.sign(
                im1[:, :, 1 : H + 1, :],
                x_t.rearrange("p cb (h w) -> p cb h w", h=H),
            )
            # kx=0: right-shift (left pad col enters at j=0)
            nc.vector.tensor_copy(
                im0[:, :, 1 : H + 1, 1:H], im1[:, :, 1 : H + 1, 0 : H - 1]
            )
            # kx=2: left-shift (right pad col at j=H-1)
            nc.vector.tensor_copy(
                im2[:, :, 1 : H + 1, 0 : H - 1], im1[:, :, 1 : H + 1, 1:H]
            )
            ims = [im0, im1, im2]
            for ob in range(OB):
                for t in range(T):
                    ps = psumpool.tile([P, NT], F32, name="cps", tag="cps", bufs=4)
                    for kk in range(KS * KS):
                        ky, kx = divmod(kk, KS)
                        rhs = ims[kx][:, :, t * R + ky : t * R + ky + R, :]
                        nc.tensor.matmul(
                            ps,
                            wT2[:, kk, ob, :, :],
                            rhs,
                            start=(kk == 0),
                            stop=(kk == KS * KS - 1),
                            perf_mode=DR,
                        )
                    sl = slice(t * NT, (t + 1) * NT)
                    tmp = tpool.tile([P, NT], F32, name="tmp")
                    nc.scalar.mul(tmp, ps, a_t[:, ob : ob + 1])
                    ot = opool.tile([P, NT], F32, name="ot")
                    nc.vector.tensor_mul(ot, tmp, bg_bcast[:, sl])
                    nc.sync.dma_start(out_v[b, ob][:, sl], ot)


def build_nc(BL):
    nc = bacc.Bacc("TRN2", target_bir_lowering=False, debug=False)
    x = nc.dram_tensor("x", [BL, C, H, H], F32, kind="ExternalInput")
    w = nc.dram_tensor("weight", [C, C, KS, KS], F32, kind="ExternalInput")
    a = nc.dram_tensor("alpha", [C, 1, 1], F32, kind="ExternalInput")
    be = nc.dram_tensor("beta", [1, H, 1], F32, kind="ExternalInput")
    g = nc.dram_tensor("gamma", [1, 1, H], F32, kind="ExternalInput")
    o = nc.dram_tensor("out", [BL, C, H, H], F32, kind="ExternalOutput")
    with tile.TileContext(nc) as tc:
        build_conv(tc, o.ap(), x.ap(), w.ap(), a.ap(), be.ap(), g.ap(), BL)
    nc.compile()
    return nc


_nc_cache = {}


def _get_nc(BL):
    if BL not in _nc_cache:
        _nc_cache[BL] = build_nc(BL)
    return _nc_cache[BL]


def kernel(x, weight, alpha, beta, gamma):
    x = np.ascontiguousarray(np.asarray(x, dtype=np.float32))
    weight = np.ascontiguousarray(np.asarray(weight, dtype=np.float32))
    alpha = np.ascontiguousarray(np.asarray(alpha, dtype=np.float32))
    beta = np.ascontiguousarray(np.asarray(beta, dtype=np.float32))
    gamma = np.ascontiguousarray(np.asarray(gamma, dtype=np.float32))

    BL = B // N_CORES
    nc = _get_nc(BL)
    xs = x.reshape(N_CORES, BL, C, H, H)
    in_maps = [
        {"x": xs[c], "weight": weight, "alpha": alpha, "beta": beta, "gamma": gamma}
        for c in range(N_CORES)
    ]
    res = run_bass_kernel_spmd(nc, in_maps, list(range(N_CORES)))
    return np.concatenate([r["out"] for r in res.results], axis=0)



# revision 2
# speedup vs baseline: 1.0896x; 1.0896x over previous
"""XNOR-Net++ 3x3 conv (sign(x) (*) sign(w) * alpha*beta*gamma) on 8 TRN2 NeuronCores.

Sharding: data-parallel over batch (32 -> 4 per core), weights/scales replicated.

Per core:
- binarize x and w on-device to fp8e4 (+-1 is exact; PSUM accumulates fp32 exactly)
- ONE width+height padded sign image per slot [128, 2, 58, 58] fp8 (two persistent
  ping-pong slots, borders zeroed once); the 3 kx taps are column offsets in the
  moving AP, so no shifted copies and no per-image memsets
- 3x3 conv = 9 accumulating DoubleRow matmuls per [128, 448] output tile
  (K=256 via input-channel-block pairing, 2 fp8 weights/PE cell)
- weights transposed on-device via PE transpose; pair dim step 128 B (%16==0)
- epilogue: single DVE mul with precomputed abg[p, ob, pix] = alpha*beta*gamma
  (alpha folded into the beta*gamma broadcast via K=1 matmuls)
- output batched per (image, ob) into SBUF, then one 1.6 MB contiguous DMA
"""

from contextlib import ExitStack

import numpy as np

import concourse.bacc as bacc
import concourse.bass as bass
import concourse.mybir as mybir
import concourse.tile as tile
from concourse import masks
from concourse.bass_utils import run_bass_kernel_spmd

N_CORES = 8
B, C, H, KS = 32, 256, 56, 3
P = 128
CB = C // P  # input-channel blocks (2)
OB = C // P  # output-channel blocks (2)
HP = H + 2   # padded image rows (58)
WP = H + 2   # padded image cols (58)
R = 8        # output rows per matmul tile
T = H // R   # row tiles per image (7)
NT = R * H   # moving free dim per matmul (448)
HW = H * H   # pixels per image (3136)

F32 = mybir.dt.float32
BF16 = mybir.dt.bfloat16
FP8 = mybir.dt.float8e4
DR = mybir.MatmulPerfMode.DoubleRow


def build_conv(tc, out_ap, x_ap, w_ap, a_ap, b_ap, g_ap, BL):
    nc = tc.nc
    with ExitStack() as ctx:
        const_pool = ctx.enter_context(tc.tile_pool(name="const", bufs=1))
        wpool = ctx.enter_context(tc.tile_pool(name="w", bufs=1))
        xpool = ctx.enter_context(tc.tile_pool(name="x", bufs=2))
        psumpool = ctx.enter_context(tc.tile_pool(name="psum", bufs=4, space="PSUM"))
        opool = ctx.enter_context(tc.tile_pool(name="o", bufs=4))

        ident = const_pool.tile([P, P], BF16, name="ident")
        masks.make_identity(nc, ident)

        # ---- scales: abg[p, ob, pix] = alpha[ob*128+p] * beta[i] * gamma[j] ----
        a_row = const_pool.tile([1, C], F32, name="a_row")
        nc.sync.dma_start(a_row, a_ap.rearrange("c u v -> (u v) c"))
        b_t = const_pool.tile([1, H], F32, name="b_t")
        nc.sync.dma_start(b_t, b_ap[0:1, :, 0])
        g_t = const_pool.tile([1, H], F32, name="g_t")
        nc.sync.dma_start(g_t, g_ap[0:1, 0, :])

        # bg_row[0, i*56+j] = beta[i] * gamma[j] — one DVE op, step-0 broadcasts
        bg_row = const_pool.tile([1, HW], F32, name="bg_row")
        b_rep = b_t[0:1, :].unsqueeze(2).to_broadcast((1, H, H))
        g_rep = g_t[0:1, :].unsqueeze(1).to_broadcast((1, H, H))
        nc.vector.tensor_mul(bg_row.rearrange("a (i j) -> a i j", i=H), b_rep, g_rep)

        # outer product alpha x bg via K=1 matmuls (before w transposes: PE can
        # run these while the big weight DMA is still landing)
        abg = const_pool.tile([P, OB, HW], F32, name="abg")
        for ob in range(OB):
            for t in range(T):
                sl = slice(t * NT, (t + 1) * NT)
                bgp = psumpool.tile([P, NT], F32, name="bgp", tag="bgp", bufs=2)
                nc.tensor.matmul(
                    bgp,
                    a_row[0:1, ob * P : (ob + 1) * P],
                    bg_row[0:1, sl],
                    start=True,
                    stop=True,
                )
                nc.scalar.copy(abg[:, ob, sl], bgp)

        # ---- weights: load, binarize, transpose, convert to fp8 ----
        w_f32 = wpool.tile([P, OB, C * KS * KS], F32, name="w_f32")
        nc.sync.dma_start(
            w_f32, w_ap.rearrange("(ob p) i ky kx -> p ob (i ky kx)", p=P)
        )
        w_sgn = wpool.tile([P, OB, C * KS * KS], BF16, name="w_sgn")
        nc.scalar.sign(w_sgn, w_f32)
        w_view = w_sgn.rearrange("p ob (i kk) -> p ob kk i", kk=KS * KS)

        # wT2[i_low, tap, ob, cb, o] in fp8; pair dim cb has byte-step 128 (%16==0)
        wT2 = wpool.tile([P, KS * KS, OB, CB, P], FP8, name="wT2")
        for ob in range(OB):
            for ib in range(CB):
                for kk in range(KS * KS):
                    pt = psumpool.tile([P, P], BF16, name="pt", tag="pt", bufs=2)
                    nc.tensor.transpose(
                        pt, w_view[:, ob, kk, ib * P : (ib + 1) * P], ident
                    )
                    nc.scalar.copy(wT2[:, kk, ob, ib, :], pt)

        # ---- persistent padded sign-image slots; borders zeroed once ----
        imgs = [
            wpool.tile([P, CB, HP, WP], FP8, name=f"img{s}") for s in range(2)
        ]
        nc.gpsimd.memset(imgs[0], 0.0)
        nc.gpsimd.memset(imgs[1], 0.0)

        # ---- main loop over local batches ----
        x_v = x_ap.rearrange("b (cb p) h w -> b p cb (h w)", p=P)
        out_v = out_ap.rearrange("b (ob p) h w -> b ob p (h w)", p=P)
        for b in range(BL):
            x_t = xpool.tile([P, CB, HW], F32, name="x_t")
            nc.sync.dma_start(x_t, x_v[b])
            im = imgs[b % 2]
            nc.scalar.sign(
                im[:, :, 1 : H + 1, 1 : H + 1],
                x_t.rearrange("p cb (h w) -> p cb h w", h=H),
            )
            for ob in range(OB):
                o_t = opool.tile([P, HW], F32, name="o_t")
                for t in range(T):
                    ps = psumpool.tile([P, NT], F32, name="cps", tag="cps", bufs=4)
                    for kk in range(KS * KS):
                        ky, kx = divmod(kk, KS)
                        rhs = im[:, :, t * R + ky : t * R + ky + R, kx : kx + H]
                        nc.tensor.matmul(
                            ps,
                            wT2[:, kk, ob, :, :],
                            rhs,
                            start=(kk == 0),
                            stop=(kk == KS * KS - 1),
                            perf_mode=DR,
                        )
                    sl = slice(t * NT, (t + 1) * NT)
                    nc.vector.tensor_mul(o_t[:, sl], ps, abg[:, ob, sl])
                nc.sync.dma_start(out_v[b, ob], o_t)


def build_nc(BL):
    nc = bacc.Bacc("TRN2", target_bir_lowering=False, debug=False)
    x = nc.dram_tensor("x", [BL, C, H, H], F32, kind="ExternalInput")
    w = nc.dram_tensor("weight", [C, C, KS, KS], F32, kind="ExternalInput")
    a = nc.dram_tensor("alpha", [C, 1, 1], F32, kind="ExternalInput")
    be = nc.dram_tensor("beta", [1, H, 1], F32, kind="ExternalInput")
    g = nc.dram_tensor("gamma", [1, 1, H], F32, kind="ExternalInput")
    o = nc.dram_tensor("out", [BL, C, H, H], F32, kind="ExternalOutput")
    with tile.TileContext(nc) as tc:
        build_conv(tc, o.ap(), x.ap(), w.ap(), a.ap(), be.ap(), g.ap(), BL)
    nc.compile()
    return nc


_nc_cache = {}


def _get_nc(BL):
    if BL not in _nc_cache:
        _nc_cache[BL] = build_nc(BL)
    return _nc_cache[BL]


def kernel(x, weight, alpha, beta, gamma):
    x = np.ascontiguousarray(np.asarray(x, dtype=np.float32))
    weight = np.ascontiguousarray(np.asarray(weight, dtype=np.float32))
    alpha = np.ascontiguousarray(np.asarray(alpha, dtype=np.float32))
    beta = np.ascontiguousarray(np.asarray(beta, dtype=np.float32))
    gamma = np.ascontiguousarray(np.asarray(gamma, dtype=np.float32))

    BL = B // N_CORES
    nc = _get_nc(BL)
    xs = x.reshape(N_CORES, BL, C, H, H)
    in_maps = [
        {"x": xs[c], "weight": weight, "alpha": alpha, "beta": beta, "gamma": gamma}
        for c in range(N_CORES)
    ]
    res = run_bass_kernel_spmd(nc, in_maps, list(range(N_CORES)))
    return np.concatenate([r["out"] for r in res.results], axis=0)


# revision 3
# speedup vs baseline: 1.2296x; 1.1284x over previous
"""XNOR-Net++ 3x3 conv (sign(x) (*) sign(w) * alpha*beta*gamma) on 8 TRN2 NeuronCores.

Sharding: data-parallel over batch (32 -> 4 per core), weights/scales replicated.

Per core:
- binarize x and w on-device to fp8e4 (+-1 is exact; PSUM accumulates fp32 exactly)
- ONE width+height padded sign image per slot [128, 2, 58, 58] fp8 (two persistent
  ping-pong slots, borders zeroed once); the 3 kx taps are column offsets in the
  moving AP, so no shifted copies and no per-image memsets
- 3x3 conv = 9 accumulating DoubleRow matmuls per [128, 448] output tile
  (K=256 via input-channel-block pairing, 2 fp8 weights/PE cell)
- weights transposed on-device via PE transpose; pair dim step 128 B (%16==0)
- epilogue: single DVE mul with precomputed abg[p, ob, pix] = alpha*beta*gamma
  (alpha folded into the beta*gamma broadcast via K=1 matmuls)
- output batched per (image, ob) into SBUF, then one 1.6 MB contiguous DMA
"""

from contextlib import ExitStack

import numpy as np

import concourse.bacc as bacc
import concourse.bass as bass
import concourse.mybir as mybir
import concourse.tile as tile
from concourse import masks
from concourse.bass_utils import run_bass_kernel_spmd

N_CORES = 8
B, C, H, KS = 32, 256, 56, 3
P = 128
CB = C // P  # input-channel blocks (2)
OB = C // P  # output-channel blocks (2)
HP = H + 2   # padded image rows (58)
WP = H + 2   # padded image cols (58)
R = 8        # output rows per matmul tile
T = H // R   # row tiles per image (7)
NT = R * H   # moving free dim per matmul (448)
HW = H * H   # pixels per image (3136)

F32 = mybir.dt.float32
BF16 = mybir.dt.bfloat16
FP8 = mybir.dt.float8e4
DR = mybir.MatmulPerfMode.DoubleRow


def build_conv(tc, out_ap, x_ap, w_ap, a_ap, b_ap, g_ap, BL):
    nc = tc.nc
    with ExitStack() as ctx:
        const_pool = ctx.enter_context(tc.tile_pool(name="const", bufs=1))
        wpool = ctx.enter_context(tc.tile_pool(name="w", bufs=1))
        xpool = ctx.enter_context(tc.tile_pool(name="x", bufs=2))
        psumpool = ctx.enter_context(tc.tile_pool(name="psum", bufs=4, space="PSUM"))
        opool = ctx.enter_context(tc.tile_pool(name="o", bufs=4))

        ident = const_pool.tile([P, P], BF16, name="ident")
        masks.make_identity(nc, ident)

        # ---- weights first: the big DMA + sign gate the PE transposes ----
        w_f32 = wpool.tile([P, OB, C * KS * KS], F32, name="w_f32")
        nc.sync.dma_start(
            w_f32, w_ap.rearrange("(ob p) i ky kx -> p ob (i ky kx)", p=P)
        )
        w_sgn = wpool.tile([P, OB, C * KS * KS], BF16, name="w_sgn")
        nc.scalar.sign(w_sgn, w_f32)
        w_view = w_sgn.rearrange("p ob (i kk) -> p ob kk i", kk=KS * KS)

        # ---- scales: abg[p, ob, pix] = alpha[ob*128+p] * beta[i] * gamma[j] ----
        a_t = const_pool.tile([P, OB], F32, name="a_t")
        nc.sync.dma_start(a_t, a_ap.rearrange("(ob p) u v -> p (ob u v)", p=P))
        b_t = const_pool.tile([1, H], F32, name="b_t")
        nc.sync.dma_start(b_t, b_ap[0:1, :, 0])
        g_t = const_pool.tile([1, H], F32, name="g_t")
        nc.sync.dma_start(g_t, g_ap[0:1, 0, :])
        ones_t = const_pool.tile([1, P], F32, name="ones_t")
        nc.gpsimd.memset(ones_t, 1.0)

        # broadcast beta/gamma rows to all 128 partitions via tiny K=1 matmuls,
        # then build abg entirely on the (otherwise idle) DVE
        b_bcast = const_pool.tile([P, H], F32, name="b_bcast")
        g_bcast = const_pool.tile([P, H], F32, name="g_bcast")
        for src, dst in ((b_t, b_bcast), (g_t, g_bcast)):
            bgp = psumpool.tile([P, H], F32, name="bgp", tag="bgp", bufs=1)
            nc.tensor.matmul(bgp, ones_t, src[0:1, :], start=True, stop=True)
            nc.vector.tensor_copy(dst, bgp)

        abg = const_pool.tile([P, OB, HW], F32, name="abg")
        abg_v = abg.rearrange("p o (i j) -> p o i j", i=H)
        ab = const_pool.tile([P, OB, H], F32, name="ab")
        for ob in range(OB):
            nc.vector.tensor_mul(
                ab[:, ob, :], b_bcast, a_t[:, ob : ob + 1].to_broadcast((P, H))
            )
            nc.vector.tensor_mul(
                abg_v[:, ob],
                ab[:, ob, :].unsqueeze(2).to_broadcast((P, H, H)),
                g_bcast.unsqueeze(1).to_broadcast((P, H, H)),
            )

        # wT2[i_low, tap, ob, cb, o] in fp8; pair dim cb has byte-step 128 (%16==0)
        wT2 = wpool.tile([P, KS * KS, OB, CB, P], FP8, name="wT2")
        for ob in range(OB):
            for ib in range(CB):
                for kk in range(KS * KS):
                    pt = psumpool.tile([P, P], BF16, name="pt", tag="pt", bufs=3)
                    nc.tensor.transpose(
                        pt, w_view[:, ob, kk, ib * P : (ib + 1) * P], ident
                    )
                    nc.scalar.copy(wT2[:, kk, ob, ib, :], pt)

        # ---- persistent padded sign-image slots; borders zeroed once ----
        imgs = [
            wpool.tile([P, CB, HP, WP], FP8, name=f"img{s}") for s in range(2)
        ]
        nc.gpsimd.memset(imgs[0], 0.0)
        nc.gpsimd.memset(imgs[1], 0.0)

        # ---- main loop over local batches ----
        x_v = x_ap.rearrange("b (cb p) h w -> b p cb (h w)", p=P)
        out_v = out_ap.rearrange("b (ob p) h w -> b ob p (h w)", p=P)
        for b in range(BL):
            x_t = xpool.tile([P, CB, HW], F32, name="x_t")
            nc.sync.dma_start(x_t, x_v[b])
            im = imgs[b % 2]
            nc.scalar.sign(
                im[:, :, 1 : H + 1, 1 : H + 1],
                x_t.rearrange("p cb (h w) -> p cb h w", h=H),
            )
            for ob in range(OB):
                o_t = opool.tile([P, HW], F32, name="o_t")
                for t in range(T):
                    ps = psumpool.tile([P, NT], F32, name="cps", tag="cps", bufs=4)
                    for kk in range(KS * KS):
                        ky, kx = divmod(kk, KS)
                        rhs = im[:, :, t * R + ky : t * R + ky + R, kx : kx + H]
                        nc.tensor.matmul(
                            ps,
                            wT2[:, kk, ob, :, :],
                            rhs,
                            start=(kk == 0),
                            stop=(kk == KS * KS - 1),
                            perf_mode=DR,
                        )
                    sl = slice(t * NT, (t + 1) * NT)
                    nc.vector.tensor_mul(o_t[:, sl], ps, abg[:, ob, sl])
                nc.sync.dma_start(out_v[b, ob], o_t)


def build_nc(BL):
    nc = bacc.Bacc("TRN2", target_bir_lowering=False, debug=False)
    x = nc.dram_tensor("x", [BL, C, H, H], F32, kind="ExternalInput")
    w = nc.dram_tensor("weight", [C, C, KS, KS], F32, kind="ExternalInput")
    a = nc.dram_tensor("alpha", [C, 1, 1], F32, kind="ExternalInput")
    be = nc.dram_tensor("beta", [1, H, 1], F32, kind="ExternalInput")
    g = nc.dram_tensor("gamma", [1, 1, H], F32, kind="ExternalInput")
    o = nc.dram_tensor("out", [BL, C, H, H], F32, kind="ExternalOutput")
    with tile.TileContext(nc) as tc:
        build_conv(tc, o.ap(), x.ap(), w.ap(), a.ap(), be.ap(), g.ap(), BL)
    nc.compile()
    return nc


_nc_cache = {}


def _get_nc(BL):
    if BL not in _nc_cache:
        _nc_cache[BL] = build_nc(BL)
    return _nc_cache[BL]


def kernel(x, weight, alpha, beta, gamma):
    x = np.ascontiguousarray(np.asarray(x, dtype=np.float32))
    weight = np.ascontiguousarray(np.asarray(weight, dtype=np.float32))
    alpha = np.ascontiguousarray(np.asarray(alpha, dtype=np.float32))
    beta = np.ascontiguousarray(np.asarray(beta, dtype=np.float32))
    gamma = np.ascontiguousarray(np.asarray(gamma, dtype=np.float32))

    BL = B // N_CORES
    nc = _get_nc(BL)
    xs = x.reshape(N_CORES, BL, C, H, H)
    in_maps = [
        {"x": xs[c], "weight": weight, "alpha": alpha, "beta": beta, "gamma": gamma}
        for c in range(N_CORES)
    ]
    res = run_bass_kernel_spmd(nc, in_maps, list(range(N_CORES)))
    return np.concatenate([r["out"] for r in res.results], axis=0)
